# revision 10
# baseline (speedup 1.0000x reference)
"""Trainium2 Bass kernel for ConsciousnessFiberAttention.

Shapes (hardcoded): b=1, s=2048, D=2048, H=16 heads, dk=128, FIBER=64.
Sharding: tensor-parallel over heads, 2 heads per core on 8 cores.
wq/wk/wv column-parallel (each core projects its own 256 q/k/v dims),
wo row-parallel (each core emits a partial (2048,2048) output; the host
sums the 8 partials and adds wo_b). The tiny fiber metric pipeline is
replicated on every core.

Per-core dataflow (all matmuls in float32r):
  phase 1: PE-transpose x into x^T tiles; project Q^T, K^T (transposed,
           scaled), V (natural), fp^T (fiber, with the curvature row
           folded in as an extra output row via w_curv = fiber_w.T @
           diag(curvature)).
  phase 1.5: fbar = mean_s fp (DVE reduce), tiny metric MLP on PE/ACT,
           M' = 0.1*outer(raw,raw) + 0.01*I, fpM^T = M' @ fp^T.
  phase 2: per (head, sq-chunk): scores^T accumulated in PSUM as
           K^T.T@Q^T/scale  +  [fp^T;curv^T].T @ [fpM^T;0.1*ones]
           (the second matmul adds COUPLING*(mod + 1*curv^T) exactly),
           ACT-exp eviction -> P^T, ones-matmul rowsums, P^T @ V ->
           out^T, normalized at eviction with a partition-broadcast
           reciprocal.
  phase 3: out^T tiles are directly the lhsT of the wo projection;
           partial = out_core @ wo_seg^T streamed to DRAM.
ci.mean() is reconstructed on the host from fbar/raw (64-dim vectors
computed on device): mean(mod)+mean(curv) = (raw.fbar)^2 + 0.1|fbar|^2
+ mean(curv).
"""
import sys

if "/opt/trn_rl_repo" not in sys.path:
    sys.path.insert(0, "/opt/trn_rl_repo")

from contextlib import ExitStack

import numpy as np

import concourse.bacc as bacc
import concourse.mybir as mybir
from concourse import tile
from concourse.bass_utils import run_bass_kernel_spmd

S = 2048
D = 2048
H = 16
DK = 128
F = 64
NCORES = 8
HPC = H // NCORES          # heads per core = 2
DSEG = HPC * DK            # 256 projected dims per core
SCALE = float(np.sqrt(DK))
COUPLING = 0.1

F32 = mybir.dt.float32
F32R = mybir.dt.float32r
AF = mybir.ActivationFunctionType

NQ = 4                     # s quarters
SQ = S // NQ               # 512
NC16 = S // 128            # 16 chunks of 128
ND = D // 128              # 16 d-chunks

_CACHE = {}


def _build():
    nc = bacc.Bacc("TRN2", target_bir_lowering=False, debug=False)

    # ---- DRAM tensors ----
    x = nc.dram_tensor("x", [S, D], F32R, kind="ExternalInput").ap()
    wqT = nc.dram_tensor("wqT", [D, DSEG], F32R, kind="ExternalInput").ap()
    wkT = nc.dram_tensor("wkT", [D, DSEG], F32R, kind="ExternalInput").ap()
    wvT = nc.dram_tensor("wvT", [D, DSEG], F32R, kind="ExternalInput").ap()
    fibT = nc.dram_tensor("fibT", [D, F + 1], F32R, kind="ExternalInput").ap()
    woT = nc.dram_tensor("woT", [DSEG, D], F32R, kind="ExternalInput").ap()
    fbias = nc.dram_tensor("fbias", [F + 1, 1], F32, kind="ExternalInput").ap()
    fm1T = nc.dram_tensor("fm1T", [F, F // 2], F32R, kind="ExternalInput").ap()
    fm2T = nc.dram_tensor("fm2T", [F // 2, F], F32R, kind="ExternalInput").ap()
    fm1b = nc.dram_tensor("fm1b", [F // 2, 1], F32, kind="ExternalInput").ap()
    fm2b = nc.dram_tensor("fm2b", [F, 1], F32, kind="ExternalInput").ap()
    id64 = nc.dram_tensor("id64", [F, F], F32R, kind="ExternalInput").ap()
    i001 = nc.dram_tensor("i001", [F, F], F32, kind="ExternalInput").ap()
    id128 = nc.dram_tensor("id128", [128, 128], F32R, kind="ExternalInput").ap()
    ones128 = nc.dram_tensor("ones128", [128, 128], F32R, kind="ExternalInput").ap()
    row01 = nc.dram_tensor("row01", [1, SQ], F32R, kind="ExternalInput").ap()

    partial = nc.dram_tensor("partial", [S, D], F32, kind="ExternalOutput").ap()
    fbar_out = nc.dram_tensor("fbar_out", [F + 1, 1], F32R, kind="ExternalOutput").ap()
    raw_out = nc.dram_tensor("raw_out", [1, F], F32R, kind="ExternalOutput").ap()

    with tile.TileContext(nc) as tc:
        with ExitStack() as outer:
            persist = outer.enter_context(tc.tile_pool(name="persist", bufs=1))

            def ptile(shape, dtype, tag):
                return persist.tile(shape, dtype, tag=tag, name=tag)

            # consts
            c_id128 = ptile([128, 128], F32R, "id128")
            nc.sync.dma_start(c_id128[:], id128)
            c_ones = ptile([128, 128], F32R, "ones128")
            nc.sync.dma_start(c_ones[:], ones128)
            c_fbias = ptile([F + 1, 1], F32, "fbias")
            nc.sync.dma_start(c_fbias[:], fbias)
            c_fm1T = ptile([F, F // 2], F32R, "fm1T")
            nc.sync.dma_start(c_fm1T[:], fm1T)
            c_fm2T = ptile([F // 2, F], F32R, "fm2T")
            nc.sync.dma_start(c_fm2T[:], fm2T)
            c_fm1b = ptile([F // 2, 1], F32, "fm1b")
            nc.sync.dma_start(c_fm1b[:], fm1b)
            c_fm2b = ptile([F, 1], F32, "fm2b")
            nc.sync.dma_start(c_fm2b[:], fm2b)
            c_id64 = ptile([F, F], F32R, "id64")
            nc.sync.dma_start(c_id64[:], id64)
            c_i001 = ptile([F, F], F32, "i001")
            nc.sync.dma_start(c_i001[:], i001)

            # persistent activations
            qT = [[ptile([128, SQ], F32R, f"qT{h}_{q}") for q in range(NQ)]
                  for h in range(HPC)]
            kT = [[ptile([128, SQ], F32R, f"kT{h}_{q}") for q in range(NQ)]
                  for h in range(HPC)]
            vN = [ptile([128, DSEG], F32R, f"v{r}") for r in range(NC16)]
            fpT = [ptile([F + 1, SQ], F32R, f"fpT{q}") for q in range(NQ)]
            fpM1 = [ptile([F + 1, SQ], F32R, f"fpM{q}") for q in range(NQ)]
            outT = [[ptile([128, SQ], F32R, f"outT{h}_{q}") for q in range(NQ)]
                    for h in range(HPC)]

            # phase 1: x^T transposes + projections (weights streamed)
            with ExitStack() as ph1:
                wfib = ph1.enter_context(tc.tile_pool(name="wfib", bufs=1))
                fibTt = [wfib.tile([128, F + 1], F32R, tag=f"fb{c}", name=f"fb{c}") for c in range(ND)]
                for c in range(ND):
                    nc.sync.dma_start(fibTt[c][:], fibT[128 * c:128 * (c + 1), :])

                xnat = ph1.enter_context(tc.tile_pool(name="xnat", bufs=5))
                xtp = ph1.enter_context(tc.tile_pool(name="xt", bufs=17))
                wqkp = ph1.enter_context(tc.tile_pool(name="wqk", bufs=4))
                wvp = ph1.enter_context(tc.tile_pool(name="wv", bufs=17))
                tps = ph1.enter_context(
                    tc.tile_pool(name="tps", bufs=2, space="PSUM"))
                pps = ph1.enter_context(
                    tc.tile_pool(name="pps", bufs=2, space="PSUM"))
                fpps = ph1.enter_context(
                    tc.tile_pool(name="fpps", bufs=1, space="PSUM"))
                vps = ph1.enter_context(
                    tc.tile_pool(name="vps", bufs=2, space="PSUM"))

                for q in range(NQ):
                    # load the 4 natural x row-tiles of this s-quarter
                    xn = []
                    for r in range(4):
                        t = xnat.tile([128, D], F32R)
                        nc.sync.dma_start(
                            t[:], x[SQ * q + 128 * r: SQ * q + 128 * (r + 1), :])
                        xn.append(t)
                    # transpose to x^T tiles (d-chunk partition, s-quarter free)
                    xt = []
                    for c in range(ND):
                        ps = tps.tile([128, SQ], F32R)
                        for r in range(4):
                            nc.tensor.transpose(
                                ps[:, 128 * r:128 * (r + 1)],
                                xn[r][:, 128 * c:128 * (c + 1)],
                                c_id128[:])
                        t = xtp.tile([128, SQ], F32R)
                        if c % 2 == 0:
                            nc.vector.tensor_copy(t[:], ps[:])
                        else:
                            nc.scalar.activation(t[:], ps[:], AF.Copy)
                        xt.append(t)

                    # transposed Q^T / K^T: both heads accumulate while the
                    # weight tile streams through once
                    for w_ap, dst, is_q in ((wqT, qT, True), (wkT, kT, False)):
                        psh = [pps.tile([128, SQ], F32, tag="ps", name="ps")
                               for _ in range(HPC)]
                        for c in range(ND):
                            t = wqkp.tile([128, DSEG], F32R)
                            nc.sync.dma_start(t[:], w_ap[128 * c:128 * (c + 1), :])
                            for h in range(HPC):
                                nc.tensor.matmul(
                                    psh[h][:], t[:, 128 * h:128 * (h + 1)], xt[c][:],
                                    start=(c == 0), stop=(c == ND - 1))
                        for h in range(HPC):
                            if is_q:
                                nc.scalar.activation(dst[h][q][:], psh[h][:],
                                                     AF.Copy, scale=1.0 / SCALE)
                            else:
                                nc.vector.tensor_copy(dst[h][q][:], psh[h][:])
                    ps = fpps.tile([F + 1, SQ], F32, tag="fppsum", name="fppsum")
                    for c in range(ND):
                        nc.tensor.matmul(ps[:], fibTt[c][:], xt[c][:],
                                         start=(c == 0), stop=(c == ND - 1))
                    nc.scalar.activation(fpT[q][:], ps[:], AF.Identity,
                                         bias=c_fbias[:])

                    # natural V projection (x^T stationary, wv^T moving)
                    wv = []
                    for c in range(ND):
                        t = wvp.tile([128, DSEG], F32R, tag="wv", name="wv")
                        nc.sync.dma_start(t[:], wvT[128 * c:128 * (c + 1), :])
                        wv.append(t)
                    for r in range(4):
                        ps = vps.tile([128, DSEG], F32)
                        for c in range(ND):
                            nc.tensor.matmul(
                                ps[:], xt[c][:, 128 * r:128 * (r + 1)], wv[c][:],
                                start=(c == 0), stop=(c == ND - 1))
                        nc.vector.tensor_copy(vN[4 * q + r][:], ps[:])

            # ---- phase 1.5: fbar, metric MLP, fpM^T ----
            with ExitStack() as ph15:
                mp = ph15.enter_context(tc.tile_pool(name="mlp", bufs=1))
                mps = ph15.enter_context(
                    tc.tile_pool(name="mlpps", bufs=1, space="PSUM"))
                red = [mp.tile([F + 1, 1], F32, tag=f"red{q}", name=f"red{q}") for q in range(NQ)]
                for q in range(NQ):
                    nc.vector.reduce_sum(red[q][:], fpT[q][:],
                                         axis=mybir.AxisListType.X)
                nc.vector.tensor_add(red[0][:], red[0][:], red[1][:])
                nc.vector.tensor_add(red[2][:], red[2][:], red[3][:])
                nc.vector.tensor_add(red[0][:], red[0][:], red[2][:])
                fbarT = mp.tile([F + 1, 1], F32R, tag="fbarT", name="fbarT")
                nc.scalar.activation(fbarT[:], red[0][:], AF.Copy, scale=1.0 / S)
                nc.sync.dma_start(fbar_out, fbarT[:])

                fb4 = mp.tile([F, 4], F32R, tag="fb4", name="fb4")
                for j in range(4):
                    nc.vector.tensor_copy(fb4[:, j:j + 1], fbarT[0:F, :])
                ps = mps.tile([F // 2, 4], F32, tag="h", name="hps")
                nc.tensor.matmul(ps[:], c_fm1T[:], fb4[:], start=True, stop=True)
                hT = mp.tile([F // 2, 4], F32R, tag="hT", name="hT")
                nc.scalar.activation(hT[:], ps[:], AF.Relu, bias=c_fm1b[:])

                ps = mps.tile([F, 4], F32, tag="raw", name="rawps")
                nc.tensor.matmul(ps[:], c_fm2T[:], hT[:], start=True, stop=True)
                rawT = mp.tile([F, 4], F32R, tag="rawT", name="rawT")
                nc.scalar.activation(rawT[:], ps[:], AF.Identity, bias=c_fm2b[:])

                ps = mps.tile([4, F], F32, tag="rawrow", name="rawrowps")
                nc.tensor.matmul(ps[:], rawT[:], c_id64[:], start=True, stop=True)
                rawRow = mp.tile([1, F], F32R, tag="rawRow", name="rawRow")
                nc.scalar.activation(rawRow[:], ps[0:1, :], AF.Copy)
                nc.sync.dma_start(raw_out, rawRow[:])

                ps = mps.tile([F, F], F32, tag="m0", name="m0ps")
                nc.tensor.matmul(ps[:], rawRow[:], rawRow[:], start=True, stop=True)
                m0 = mp.tile([F, F], F32, tag="m0s", name="m0s")
                nc.scalar.activation(m0[:], ps[:], AF.Copy, scale=COUPLING)
                mprime = mp.tile([F, F], F32R, tag="mprime", name="mprime")
                nc.vector.tensor_add(mprime[:], m0[:], c_i001[:])

                for q in range(NQ):
                    ps = mps.tile([F, SQ], F32, tag="fpmps", name="fpmps")
                    nc.tensor.matmul(ps[:], mprime[:], fpT[q][0:F, :],
                                     start=True, stop=True)
                    nc.scalar.activation(fpM1[q][0:F, :], ps[:], AF.Copy)
                    nc.sync.dma_start(fpM1[q][F:F + 1, :], row01)

            # ---- phase 2: attention per (head, sq-chunk) ----
            with ExitStack() as ph2:
                ptp = ph2.enter_context(tc.tile_pool(name="pt", bufs=20))
                rbp = ph2.enter_context(tc.tile_pool(name="rb", bufs=3))
                sps = ph2.enter_context(
                    tc.tile_pool(name="sps", bufs=3, space="PSUM"))
                ops_ = ph2.enter_context(
                    tc.tile_pool(name="ops", bufs=2, space="PSUM"))
                rps = ph2.enter_context(
                    tc.tile_pool(name="rps", bufs=2, space="PSUM"))

                for h in range(HPC):
                    for q in range(NQ):
                        pO = ops_.tile([128, SQ], F32)
                        pR = rps.tile([128, SQ], F32)
                        for t in range(NC16):
                            tq, tr = divmod(t, 4)
                            pS = sps.tile([128, SQ], F32)
                            nc.tensor.matmul(
                                pS[:], kT[h][tq][:, 128 * tr:128 * (tr + 1)],
                                qT[h][q][:], start=True, stop=False)
                            nc.tensor.matmul(
                                pS[:], fpT[tq][:, 128 * tr:128 * (tr + 1)],
                                fpM1[q][:], start=False, stop=True)
                            pt = ptp.tile([128, SQ], F32R)
                            nc.scalar.activation(pt[:], pS[:], AF.Exp)
                            nc.tensor.matmul(pR[:], c_ones[:], pt[:],
                                             start=(t == 0), stop=(t == NC16 - 1))
                            nc.tensor.matmul(
                                pO[:], vN[t][:, 128 * h:128 * (h + 1)], pt[:],
                                start=(t == 0), stop=(t == NC16 - 1))
                        rb = rbp.tile([128, SQ], F32)
                        nc.vector.reciprocal(rb[:], pR[:])
                        nc.vector.tensor_mul(outT[h][q][:], pO[:], rb[:])

            # ---- phase 3: output projection (row-parallel partial) ----
            with ExitStack() as ph3:
                wop = ph3.enter_context(tc.tile_pool(name="wo", bufs=1))
                fop = ph3.enter_context(tc.tile_pool(name="fo", bufs=3))
                fps_ = ph3.enter_context(
                    tc.tile_pool(name="fps", bufs=4, space="PSUM"))
                woTt = [wop.tile([128, D], F32R, tag=f"wo{h}", name=f"wo{h}") for h in range(HPC)]
                for h in range(HPC):
                    nc.sync.dma_start(woTt[h][:],
                                      woT[128 * h:128 * (h + 1), :])
                for i in range(NC16):
                    iq, ir = divmod(i, 4)
                    fo = fop.tile([128, D], F32)
                    for dc in range(4):
                        pF = fps_.tile([128, SQ], F32)
                        for h in range(HPC):
                            nc.tensor.matmul(
                                pF[:], outT[h][iq][:, 128 * ir:128 * (ir + 1)],
                                woTt[h][:, SQ * dc:SQ * (dc + 1)],
                                start=(h == 0), stop=(h == HPC - 1))
                        nc.scalar.activation(fo[:, SQ * dc:SQ * (dc + 1)],
                                             pF[:], AF.Copy)
                    nc.sync.dma_start(partial[128 * i:128 * (i + 1), :], fo[:])

    nc.compile()
    return nc


def _prep_inputs(inputs):
    """Host-side sharding/layout prep. Returns per-core in_maps."""
    g = {k: np.asarray(v, dtype=np.float32) for k, v in inputs.items()}
    x = np.ascontiguousarray(g["x"].reshape(S, D))
    cm_diag = np.ascontiguousarray(np.diagonal(g["curvature_modulator"]))
    w_curv = g["fiber_w"].T @ cm_diag                       # (D,)
    fibT = np.ascontiguousarray(
        np.concatenate([g["fiber_w"].T, w_curv[:, None]], axis=1))  # (D, 65)
    fbias = np.concatenate(
        [g["fiber_b"], [g["fiber_b"] @ cm_diag]]).astype(np.float32)[:, None]
    fm1T = np.ascontiguousarray(g["fm1_w"].T)
    fm2T = np.ascontiguousarray(g["fm2_w"].T)
    fm1b = np.ascontiguousarray(g["fm1_b"][:, None])
    fm2b = np.ascontiguousarray(g["fm2_b"][:, None])
    id64 = np.eye(F, dtype=np.float32)
    i001 = (COUPLING * COUPLING) * np.eye(F, dtype=np.float32)
    id128 = np.eye(128, dtype=np.float32)
    ones128 = np.ones((128, 128), dtype=np.float32)
    row01c = np.full((1, SQ), COUPLING, dtype=np.float32)

    in_maps = []
    for m in range(NCORES):
        lo, hi = DSEG * m, DSEG * (m + 1)
        in_maps.append({
            "x": x,
            "wqT": np.ascontiguousarray(g["wq"][lo:hi, :].T),
            "wkT": np.ascontiguousarray(g["wk"][lo:hi, :].T),
            "wvT": np.ascontiguousarray(g["wv"][lo:hi, :].T),
            "fibT": fibT,
            "woT": np.ascontiguousarray(g["wo_w"][:, lo:hi].T),
            "fbias": fbias,
            "fm1T": fm1T,
            "fm2T": fm2T,
            "fm1b": fm1b,
            "fm2b": fm2b,
            "id64": id64,
            "i001": i001,
            "id128": id128,
            "ones128": ones128,
            "row01": row01c,
        })
    return in_maps


def run(inputs, **kw):
    """Build (cached), run on 8 cores, return (results, BassKernelResults)."""
    if "nc" not in _CACHE:
        _CACHE["nc"] = _build()
    nc = _CACHE["nc"]
    in_maps = _prep_inputs(inputs)
    res = run_bass_kernel_spmd(nc, in_maps, core_ids=list(range(NCORES)), **kw)
    return res


def _postprocess(res, inputs):
    wo_b = np.asarray(inputs["wo_b"], dtype=np.float32)
    out = np.zeros((S, D), dtype=np.float64)
    for r in res.results:
        out += np.asarray(r["partial"], dtype=np.float64)
    out = (out + wo_b[None, :]).astype(np.float32).reshape(1, S, D)

    r0 = res.results[0]
    fbar_ext = np.asarray(r0["fbar_out"], dtype=np.float32).reshape(F + 1)
    raw = np.asarray(r0["raw_out"], dtype=np.float32).reshape(F)
    fbar = fbar_ext[:F]
    ci_mean = np.float32(
        float(raw @ fbar) ** 2 + 0.1 * float(fbar @ fbar) + float(fbar_ext[F]))
    return out, ci_mean


def kernel(**inputs):
    res = run(inputs)
    return _postprocess(res, inputs)
